# revision 24
# baseline (speedup 1.0000x reference)
"""Trainium2 Bass kernel for nn_Attention_41532333753073.

Math (per batch b):
  q = X@Wq, k = X@Wk, v = lambda1*v1 + lambda2*(X@Wv)
  q,k = rope(q), rope(k)   (GPT-NeoX half-split, theta=1e4)
  o = softmax(causal(q k^T / sqrt(64))) @ v
  out = o @ Wo

Sharding: 8 cores = 2 batches x 4 head-groups (8 heads each).
Each core computes its group's partial out-projection; host sums the 4
partials per batch.

Device layout choices (per core):
  - X^T resident in SBUF (bf16); Q/K produced transposed ([qcol, tok]) with
    an A/B half-split column order so RoPE runs as full-128-partition DVE ops.
  - Scores computed transposed (ST[k, q]) per 128-key block so the PV matmul
    consumes exp(ST) directly as the moving operand (no transposes anywhere).
  - 4 heads packed in the PE array via tile_position (K=32 row strips).
  - softmax denominator via an appended ones-column in V (M=65 PV matmuls);
    reciprocal on a [16,32] reshape; broadcast via K=1 ones matmul.
  - causal masking via precomputed 0/1 bf16 masks on the 4 diagonal block
    patterns only; fully-masked blocks are skipped (causal flop skip).
"""

import numpy as np
import ml_dtypes

B, T, DM = 2, 2048, 2048
H, DH = 32, 64
N_CORES = 8
HG = 8            # heads per core-group
ROPE_THETA = 10000.0
BF = ml_dtypes.bfloat16

_CACHE = {}


# ----------------------------------------------------------------- device ---

def _declare_io(nc):
    import concourse.mybir as mybir
    BF16 = mybir.dt.bfloat16
    return dict(
        xt=nc.dram_tensor("xt", [DM, T], BF16, kind="ExternalInput").ap(),
        wqk=nc.dram_tensor("wqk", [8, 16, 128, 128], BF16,
                           kind="ExternalInput").ap(),
        wv=nc.dram_tensor("wv", [16, 128, 512], BF16,
                          kind="ExternalInput").ap(),
        wo=nc.dram_tensor("wo", [4, 128, 2048], BF16,
                          kind="ExternalInput").ap(),
        v1l=nc.dram_tensor("v1l", [T, 512], BF16, kind="ExternalInput").ap(),
        cosr=nc.dram_tensor("cosr", [128, T], BF16,
                            kind="ExternalInput").ap(),
        sinr=nc.dram_tensor("sinr", [128, T], BF16,
                            kind="ExternalInput").ap(),
        mask=nc.dram_tensor("mask", [128, 4, 512], BF16,
                            kind="ExternalInput").ap(),
        out=nc.dram_tensor("out", [T, DM], BF16, kind="ExternalOutput").ap(),
    )


def _emit(nc, tc, io, phases=(1, 2, 3, 4)):
    import concourse.mybir as mybir
    F32 = mybir.dt.float32
    BF16 = mybir.dt.bfloat16
    AF = mybir.ActivationFunctionType
    ALU = mybir.AluOpType

    xt, wqk, wv, wo, v1l = io["xt"], io["wqk"], io["wv"], io["wo"], io["v1l"]
    cosr, sinr, mask, out = io["cosr"], io["sinr"], io["mask"], io["out"]

    with tc.tile_pool(name="consts", bufs=1) as consts, \
         tc.tile_pool(name="qk_sb", bufs=1) as qk_sb, \
         tc.tile_pool(name="v_pool", bufs=1) as v_pool, \
         tc.tile_pool(name="ot_pool", bufs=1) as ot_pool:

        ones1 = consts.tile([1, 128], F32, name="ones1")
        nc.vector.memset(ones1, 1.0)

        # Persistent activation tiles
        qt = [qk_sb.tile([128, T], BF16, name=f"qt{cb}") for cb in range(4)]
        kt = [qk_sb.tile([128, T], BF16, name=f"kt{cb}") for cb in range(4)]
        v_sb = [v_pool.tile([128, HG, 65], BF16, name=f"v{m}") for m in range(16)]
        ot = [ot_pool.tile([128, T], BF16, name=f"ot{c}") for c in range(4)]

        # ---------------- Phase 1+2: projections + rope + value mix --------
        dummy_out_written = False
        with tc.tile_pool(name="xt_pool", bufs=1) as xt_pool, \
             tc.tile_pool(name="wv_pool", bufs=1) as wv_pool, \
             tc.tile_pool(name="rope", bufs=1) as rope, \
             tc.tile_pool(name="v1_stream", bufs=2) as v1_stream:

            cos_sb = rope.tile([128, T], BF16, name="cos_sb", bufs=1)
            sin_sb = rope.tile([128, T], BF16, name="sin_sb", bufs=1)
            nc.sync.dma_start(cos_sb, cosr)
            nc.sync.dma_start(sin_sb, sinr)
            xt_sb = xt_pool.tile([128, 16, T], BF16, name="xt_sb")
            xt_r = xt.rearrange("(k p) t -> p k t", p=128)
            wv_sb = wv_pool.tile([128, 16, 512], BF16, name="wv_sb")
            wv_r = wv.rearrange("k p d -> p k d")
            wqk_sb = wv_pool.tile([128, 8, 16, 128], BF16, name="wqk_sb")
            wqk_r = wqk.rearrange("c k p m -> p c k m")
            # first-needed chunks first so PE can start early
            nc.sync.dma_start(wqk_sb[:, 0, 0:2], wqk_r[:, 0, 0:2])
            nc.sync.dma_start(xt_sb[:, 0, :], xt_r[:, 0, :])
            nc.sync.dma_start(wqk_sb[:, 0, 2:16], wqk_r[:, 0, 2:16])
            for k in range(1, 16):
                nc.sync.dma_start(xt_sb[:, k, :], xt_r[:, k, :])
            for cb in range(1, 8):
                nc.sync.dma_start(wqk_sb[:, cb], wqk_r[:, cb])
            nc.sync.dma_start(wv_sb[:, 0:8], wv_r[:, 0:8])
            nc.sync.dma_start(wv_sb[:, 8:16], wv_r[:, 8:16])

            with tc.tile_pool(name="qk_psum", bufs=2, space="PSUM") as qk_psum:
              for wi, dst in ((0, qt), (1, kt)):
                for quad in range(2):
                    pab = []
                    for half in range(2):
                        cb = 2 * quad + half
                        p = qk_psum.tile([128, T], F32, tag="qkps",
                                         name=f"qkps{cb}")
                        for k in range(16):
                            for tch in range(4):
                                nc.tensor.matmul(
                                    p[:, tch * 512:(tch + 1) * 512],
                                    lhsT=wqk_sb[:, 4 * wi + cb, k, :],
                                    rhs=xt_sb[:, k, tch * 512:(tch + 1) * 512],
                                    start=(k == 0), stop=(k == 15))
                        pab.append(p)
                    qa = rope.tile([128, T], BF16, tag="qa", name="qa")
                    qb = rope.tile([128, T], BF16, tag="qb", name="qb")
                    nc.scalar.copy(qa, pab[0])
                    nc.scalar.copy(qb, pab[1])
                    dA, dB = dst[2 * quad], dst[2 * quad + 1]
                    t1 = rope.tile([128, T], BF16, tag="t1", name="t1")
                    nc.vector.tensor_tensor(dA, qa, cos_sb, ALU.mult)
                    nc.vector.tensor_tensor(t1, qb, sin_sb, ALU.mult)
                    nc.vector.tensor_tensor(dA, dA, t1, ALU.subtract)
                    nc.vector.tensor_tensor(dB, qb, cos_sb, ALU.mult)
                    nc.vector.tensor_tensor(t1, qa, sin_sb, ALU.mult)
                    nc.vector.tensor_tensor(dB, dB, t1, ALU.add)

            # V = lambda1*v1 + lambda2*(X@Wv), plus ones column for rowsum
            if 2 not in phases:
                nc.sync.dma_start(out[0:128, :], qt[0])
                return
            with tc.tile_pool(name="v_psum", bufs=2, space="PSUM") as v_psum:
                for m in range(16):
                    vp = v_psum.tile([128, 512], F32, tag="vps", name=f"vp{m}")
                    for k in range(16):
                        nc.tensor.matmul(
                            vp, lhsT=xt_sb[:, k, m * 128:(m + 1) * 128],
                            rhs=wv_sb[:, k, :],
                            start=(k == 0), stop=(k == 15))
                    v1t = v1_stream.tile([128, 512], BF16, tag="v1t",
                                         name=f"v1t{m}")
                    nc.gpsimd.dma_start(v1t, v1l[m * 128:(m + 1) * 128, :])
                    nc.vector.memset(v_sb[m][:, :, 64:65], 1.0)
                    nc.vector.tensor_tensor(
                        v_sb[m][:, :, 0:64],
                        vp.rearrange("p (h d) -> p h d", h=HG),
                        v1t.rearrange("p (h d) -> p h d", h=HG),
                        ALU.add)

        # ------- Phase 3+4: attention + inline output projection -----------
        if 3 not in phases:
            nc.sync.dma_start(out[0:128, :], qt[0])
            return
        do_p4 = 4 in phases
        tc.strict_bb_all_engine_barrier()
        with tc.tile_pool(name="e_pool", bufs=1) as e_pool, \
             tc.tile_pool(name="aux", bufs=2) as aux, \
             tc.tile_pool(name="wo_pool", bufs=1) as wo_pool, \
             tc.tile_pool(name="stage", bufs=2) as stage, \
             tc.tile_pool(name="st_psum", bufs=1, space="PSUM") as st_psum, \
             tc.tile_pool(name="st_psumb", bufs=1, space="PSUM") as st_psumb, \
             tc.tile_pool(name="ut_psum", bufs=2, space="PSUM") as ut_psum, \
             tc.tile_pool(name="bc_psum", bufs=2, space="PSUM") as bc_psum:

            tri = aux.tile([128, 128], BF16, name="tri", bufs=1)
            nc.sync.dma_start(tri, mask[:, 0, 0:128])
            wo_sb = wo_pool.tile([128, 4, 2048], BF16, name="wo_sb")
            nc.sync.dma_start(wo_sb, wo.rearrange("c p d -> p c d"))

            def pv_steps(quad, j, esbs):
                """Emit PV matmuls + softmax normalization for (quad, j);
                yields between small batches so the caller can interleave
                this PE-side work into the next iteration's ST/exp loop."""
                kbm = 4 * j + 4
                qsl = slice(j * 512, (j + 1) * 512)
                for h in range(4):
                    gh = quad * 4 + h
                    utp = ut_psum.tile([65, 512], F32, tag="ut",
                                       name=f"ut{quad}_{j}_{h}")
                    for kb in range(kbm):
                        o_ = kb - 4 * j
                        c0 = o_ * 128 if o_ >= 0 else 0
                        nc.tensor.matmul(
                            utp[:, c0:], lhsT=v_sb[kb][:, gh, :],
                            rhs=esbs[kb][:, h, c0:],
                            start=(kb == 0), stop=(kb == kbm - 1),
                            skip_group_check=True)
                        if kb % 2 == 1:
                            yield
                    # softmax denominator -> reciprocal -> broadcast -> mul
                    rrow = aux.tile([1, 512], F32, tag="rrow", name="rrow",
                                    bufs=2)
                    nc.scalar.copy(rrow, utp[64:65, :])
                    rtmp = aux.tile([16, 32], F32, tag="rtmp", name="rtmp",
                                    bufs=2)
                    nc.sync.dma_start(rtmp, rrow)
                    rinv = aux.tile([16, 32], F32, tag="rinv", name="rinv",
                                    bufs=2)
                    nc.vector.reciprocal(rinv, rtmp)
                    rri = aux.tile([1, 512], F32, tag="rri", name="rri",
                                   bufs=2)
                    nc.sync.dma_start(rri, rinv)
                    bcp = bc_psum.tile([128, 512], F32, tag="bcpp",
                                       name="bcp")
                    nc.tensor.matmul(bcp, lhsT=ones1, rhs=rri,
                                     start=True, stop=True)
                    rb = aux.tile([64, 512], BF16, tag="rb", name="rb", bufs=1)
                    nc.scalar.copy(rb, bcp[0:64, :])
                    rows = slice((gh % 2) * 64, (gh % 2) * 64 + 64)
                    nc.vector.tensor_tensor(
                        ot[gh // 2][rows, qsl], utp[0:64, :], rb, ALU.mult)
                    yield

            def wo_steps(j):
                """Output projection for token blocks of q-chunk j (needs all
                heads of chunk j normalized into ot)."""
                for m in range(4 * j, 4 * j + 4):
                    msl = slice(m * 128, (m + 1) * 128)
                    for n in range(4):
                        pp = bc_psum.tile([128, 512], F32, tag="bcpp",
                                          name=f"pp{m}_{n}")
                        for oc in range(4):
                            nc.tensor.matmul(
                                pp, lhsT=ot[oc][:, msl],
                                rhs=wo_sb[:, oc, n * 512:(n + 1) * 512],
                                start=(oc == 0), stop=(oc == 3))
                        st_out = stage.tile([128, 512], BF16, tag="st_out",
                                            name=f"st_out{m}_{n}")
                        nc.scalar.copy(st_out, pp)
                        nc.sync.dma_start(out[msl, n * 512:(n + 1) * 512],
                                          st_out)
                        yield

            def chain(gens):
                for g in gens:
                    yield from g

            pending = None
            pending_n = 0

            def step_pending(k):
                nonlocal pending, pending_n
                for _ in range(k):
                    if pending is None:
                        return
                    if next(pending, "done") == "done":
                        pending = None
                        pending_n = 0
                    else:
                        pending_n -= 1

            for j in (3, 2, 1, 0):
                kbmax = 4 * j + 4
                qsl = slice(j * 512, (j + 1) * 512)
                esbs_q = []
                for quad in range(2):
                    ktA, ktB = kt[2 * quad], kt[2 * quad + 1]
                    qtA, qtB = qt[2 * quad], qt[2 * quad + 1]
                    esbs = []
                    for kb in range(kbmax):
                        stpa = st_psum.tile([128, 1024], F32, tag="sta",
                                            name=f"sta{quad}_{j}_{kb}")
                        stpb = st_psumb.tile([128, 1024], F32, tag="stb",
                                             name=f"stb{quad}_{j}_{kb}")
                        ksl = slice(kb * 128, (kb + 1) * 128)
                        oc0 = max(kb - 4 * j, 0) * 128  # causal col skip
                        qslc = slice(j * 512 + oc0, (j + 1) * 512)
                        for half_t, st_flag in ((ktA, True), (ktB, False)):
                            rt = qtA if st_flag else qtB
                            for h in range(4):
                                hs = slice(h * 32, (h + 1) * 32)
                                dst = stpa if h < 2 else stpb
                                nc.tensor.matmul(
                                    dst[:, (h % 2) * 512 + oc0:
                                        (h % 2 + 1) * 512],
                                    lhsT=half_t[hs, ksl], rhs=rt[hs, qslc],
                                    start=st_flag, stop=not st_flag,
                                    tile_position=(h * 32, 0))
                        etag = (f"e{quad}_{kb}" if kb < 12 else
                                f"esh_{kb}")
                        e = e_pool.tile([128, 4, 512], BF16, tag=etag,
                                        name=f"e{quad}_{j}_{kb}", bufs=1)
                        o_ = kb - 4 * j
                        c0 = o_ * 128 if o_ >= 0 else 0
                        sva = stpa.rearrange("p (h q) -> p h q", h=2)
                        svb = stpb.rearrange("p (h q) -> p h q", h=2)
                        nc.scalar.activation(e[:, 0:2, c0:], sva[:, :, c0:],
                                             AF.Exp)
                        nc.scalar.activation(e[:, 2:4, c0:], svb[:, :, c0:],
                                             AF.Exp)
                        if o_ >= 0:
                            # zero sub-diagonal triangle of the diagonal block
                            nc.vector.tensor_tensor(
                                e[:, :, c0:c0 + 128], e[:, :, c0:c0 + 128],
                                tri[:, None, :].broadcast_to([128, 4, 128]),
                                ALU.mult)
                        esbs.append(e)
                        # interleave deferred PE work into the ACT-bound loop
                        slots_left = kbmax - kb
                        step_pending(max(2, pending_n // max(slots_left, 1)))
                    esbs_q.append(esbs)
                    # drain remaining deferred work, then queue this quad's
                    step_pending(10 ** 9)
                    gens = [pv_steps(quad, j, esbs)]
                    nunits = 4 * (kbmax // 2 + 1)
                    if quad == 1 and do_p4:
                        gens.append(wo_steps(j))
                        nunits += 16
                    pending = chain(gens)
                    pending_n = nunits
            step_pending(10 ** 9)
            if not do_p4:
                for c in range(4):
                    nc.sync.dma_start(out[c * 128:(c + 1) * 128, :], ot[c])


def build_nc(n_rep=1, phases=(1, 2, 3, 4)):
    """Build and schedule the full Bass program (cached). n_rep>1 replicates
    the body (for timing measurements)."""
    key = ("nc", n_rep, tuple(phases))
    if key in _CACHE:
        return _CACHE[key]
    import concourse.bass as bass
    import concourse.tile as tile
    nc = bass.Bass("TRN2", target_bir_lowering=False, debug=False)
    io = _declare_io(nc)
    with tile.TileContext(nc) as tc:
        for rep in range(n_rep):
            if rep:
                tc.strict_bb_all_engine_barrier()
            _emit(nc, tc, io, phases=phases)
    _CACHE[key] = nc
    return nc


# ------------------------------------------------------------------- host ---

def _rope_tables():
    inv_freq = 1.0 / (ROPE_THETA ** (np.arange(0, DH, 2, dtype=np.float32) / DH))
    t = np.arange(T, dtype=np.float32)
    freqs = np.outer(t, inv_freq)           # [T, 32]
    cosr = np.tile(np.cos(freqs).T, (4, 1))  # [128, T]
    sinr = np.tile(np.sin(freqs).T, (4, 1))
    return cosr.astype(BF), sinr.astype(BF)


def _mask_tables():
    p = np.arange(128)[:, None]
    c = np.arange(512)[None, :]
    m = np.zeros((128, 4, 512), np.float32)
    for o in range(4):
        m[:, o, :] = (c >= o * 128 + p)
    return m.astype(BF)


def make_core_inputs(hidden_states, v1, lambda1, Wq, Wk, Wv, Wo, lambda2):
    """Build the 8 per-core input dicts (core i = batch i//4, group i%4)."""
    cosr, sinr = _rope_tables()
    maskt = _mask_tables()
    sc = np.float32(1.0 / np.sqrt(DH))
    xts = [np.ascontiguousarray(hidden_states[b].T).astype(BF) for b in range(B)]

    def reorder_qk(W, g, scale):
        blocks = []
        for quad in range(2):
            for half in range(2):
                cols = []
                for hl in range(4):
                    gh = g * HG + quad * 4 + hl
                    c0 = gh * 64 + half * 32
                    cols.append(np.arange(c0, c0 + 32))
                cols = np.concatenate(cols)
                blk = (W[:, cols] * scale).astype(BF)      # [2048, 128]
                blocks.append(blk.reshape(16, 128, 128))
        return np.stack(blocks)                             # [4, 16, 128, 128]

    in_maps = []
    for i in range(N_CORES):
        b, g = divmod(i, 4)
        gc = slice(g * HG * DH, (g + 1) * HG * DH)
        wv_re = (Wv[:, gc] * np.float32(lambda2)).astype(BF).reshape(16, 128, 512)
        wo_re = Wo[gc, :].astype(BF).reshape(4, 128, 2048)
        v1l = (np.float32(lambda1) *
               v1[b, :, g * HG:(g + 1) * HG, :].reshape(T, 512)).astype(BF)
        in_maps.append(dict(
            xt=xts[b],
            wqk=np.concatenate([reorder_qk(Wq, g, sc),
                                reorder_qk(Wk, g, np.float32(1.0))]),
            wv=np.ascontiguousarray(wv_re),
            wo=np.ascontiguousarray(wo_re),
            v1l=v1l,
            cosr=cosr, sinr=sinr, mask=maskt,
        ))
    return in_maps


def core_reference(im):
    """Numpy reference for one core's math (for sim debugging)."""
    f = np.float32
    xt = im["xt"].astype(f)
    cos = im["cosr"].astype(f)
    sin = im["sinr"].astype(f)

    def proj_qk(wblk):
        qt = np.zeros((4, 128, T), f)
        for cb in range(4):
            w = wblk[cb].astype(f).reshape(2048, 128)
            qt[cb] = (xt.T @ w).T
        # rope: (A,B) pairs
        o = np.zeros_like(qt)
        for quad in range(2):
            A, Bb = qt[2 * quad], qt[2 * quad + 1]
            o[2 * quad] = A * cos - Bb * sin
            o[2 * quad + 1] = Bb * cos + A * sin
        return o.astype(BF).astype(f)

    qtr = proj_qk(im["wqk"][:4])
    ktr = proj_qk(im["wqk"][4:])
    wv = im["wv"].astype(f).reshape(2048, 512)
    v = (xt.T @ wv + im["v1l"].astype(f)).astype(BF).astype(f)  # [T, 512]

    out = np.zeros((T, DM), f)
    ot = np.zeros((512, T), f)
    for quad in range(2):
        for h in range(4):
            gh = quad * 4 + h
            hs = slice(h * 32, (h + 1) * 32)
            qh = np.concatenate([qtr[2 * quad][hs], qtr[2 * quad + 1][hs]])  # [64, T]
            kh = np.concatenate([ktr[2 * quad][hs], ktr[2 * quad + 1][hs]])
            st = (kh.T @ qh)                      # [keys, q]
            e = np.exp(st).astype(BF).astype(f)
            ksm = np.tril(np.ones((T, T), f))     # mask[q, k] -> e[k, q]
            e = e * ksm.T
            vh = v[:, gh * 64:(gh + 1) * 64]      # [keys, 64]
            u = vh.T @ e                          # [64, q]
            r = e.sum(axis=0)                     # [q]
            oh = (u * (1.0 / r)[None, :]).astype(BF).astype(f)
            ot[gh * 64:(gh + 1) * 64] = oh
    wo = im["wo"].astype(f).reshape(512, 2048)
    out = (ot.T @ wo).astype(BF).astype(f)
    return out


def _install_wait_legalizer():
    """This container's walrus accepts only one semaphore wait per
    instruction; split extra waits into preceding single-wait NoOps."""
    import json as _json
    from concourse import bass2jax as _b2j
    if getattr(_b2j, "_wait_legalizer_installed", False):
        return
    _orig = _b2j.compile_bir_kernel

    def _legalized(bir_json, tmpdir, neff_name="file.neff"):
        d = _json.loads(bir_json)
        n = 0
        for fn in d.get("functions", []):
            for blk in fn.get("blocks", []):
                out = []
                for inst in blk.get("instructions", []):
                    si = inst.get("sync_info")
                    waits = (si or {}).get("on_wait") or []
                    if len(waits) > 1:
                        for w in waits[:-1]:
                            n += 1
                            out.append({
                                "debug": inst.get("debug", 0),
                                "engine": inst["engine"],
                                "ins": [], "outs": [],
                                "name": f"I-legw-{n}",
                                "opcode": "NoOp",
                                "sync_info": {"on_update": [], "on_wait": [w]},
                            })
                        si["on_wait"] = [waits[-1]]
                    out.append(inst)
                blk["instructions"] = out
        return _orig(_json.dumps(d).encode(), tmpdir, neff_name=neff_name)

    _b2j.compile_bir_kernel = _legalized
    _b2j._wait_legalizer_installed = True


def kernel(hidden_states, v1, lambda1, Wq, Wk, Wv, Wo, lambda2):
    from concourse import bass_utils
    _install_wait_legalizer()
    args = (np.asarray(hidden_states, np.float32), np.asarray(v1, np.float32),
            np.float32(lambda1), np.asarray(Wq, np.float32),
            np.asarray(Wk, np.float32), np.asarray(Wv, np.float32),
            np.asarray(Wo, np.float32), np.float32(lambda2))
    in_maps = make_core_inputs(*args)
    nc = build_nc()
    res = bass_utils.run_bass_kernel_spmd(nc, in_maps,
                                          core_ids=list(range(N_CORES)))
    outp = np.zeros((B, T, DM), np.float32)
    for i in range(N_CORES):
        b = i // 4
        outp[b] += res.results[i]["out"].astype(np.float32)
    return outp
